# revision 25
# baseline (speedup 1.0000x reference)
"""Fused attention kernel for TRN2, data-parallel over 8 NeuronCores.

Problem: LN -> qk/v projections -> softplus-polar embedding -> attention
-> output projection.  B=8 batch elements are sharded one-per-core; each
core runs an identical single-core program (no collectives).

Layout strategy (per core, N=1024 tokens, D=1024, H=16 heads, DH=64):
  - LN in [n, d] layout (free-axis reductions), then PE-transpose to
    xnT [d, n] (bf16) for the projection GEMMs.
  - q/k produced TRANSPOSED ([e, n]) with w_qk tiles as the stationary
    operand; v produced natural ([n, e]) with xnT as stationary.
  - polar: softplus = Ln(1+Exp(x)) on ScalarE (Exp in-place on PSUM; the
    Ln/Exp pair lives in ONE activation table set, no table thrash), the
    64-row head slice is duplicated into both partition halves by two
    SBUF->SBUF DMAs, then one DVE multiply against a [cosT; sinT] table.
  - attention entirely in transposed layout: S^T = k2t.T @ q2t (K=128,
    single k-tile), exp via ScalarE (scale=DH^-0.5 fused), then
    O_un^T = V'.T @ E^T with V' stationary [128, 65] per (m-tile, head).
    V' carries a ones-column (last col for even heads, FIRST col for odd
    heads) so the softmax denominator L lands on psum row 64/63; O^T for
    a head pair fills one [128, n] e-tile directly -- no O transpose.
    Normalization: DVE reciprocal of the L row, partition-broadcast via
    a strided SBUF->SBUF DMA, one DVE multiply into ot_sb.
  - final GEMM: lhsT = ot tiles, rhs = w_out.

ln_gamma is folded into w_qk/w_v rows on the host; ln_beta enters as an
ACT bias ([128,1] per e-chunk) for q/k and a broadcast add for v; b_out
is a broadcast add on the output.  All exact algebra, ~zero device cost.
"""

import os

import ml_dtypes
import numpy as np

import concourse.bass as bass
import concourse.tile as tile
from concourse import bacc, mybir
from concourse.bass_utils import run_bass_kernel_spmd
from concourse.masks import make_identity

F32 = mybir.dt.float32
BF16 = mybir.dt.bfloat16
AF = mybir.ActivationFunctionType
ALU = mybir.AluOpType

B, N, D, H, DH = 8, 1024, 1024, 16, 64
NC_, DT_, EC_Q, MC_ = 8, 8, 8, 8  # n-chunks, d-tiles, q e-chunks, m-tiles
SCALE = DH ** -0.5


def _emit(tc):
    nc = tc.nc

    x_d = nc.dram_tensor("x", [N, D], F32, kind="ExternalInput").ap()
    wqk_d = nc.dram_tensor("wqk", [D, 2 * H * DH], BF16, kind="ExternalInput").ap()
    wv_d = nc.dram_tensor("wv", [D, H * DH], BF16, kind="ExternalInput").ap()
    wout_d = nc.dram_tensor("wout", [H * DH, D], BF16, kind="ExternalInput").ap()
    csq_d = nc.dram_tensor("csq", [128, N], BF16, kind="ExternalInput").ap()
    csk_d = nc.dram_tensor("csk", [128, N], BF16, kind="ExternalInput").ap()
    qbias_d = nc.dram_tensor("qbias", [128, 16], F32, kind="ExternalInput").ap()
    vbias_d = nc.dram_tensor("vbias", [1, H * DH], F32, kind="ExternalInput").ap()
    bout_d = nc.dram_tensor("bout", [1, D], F32, kind="ExternalInput").ap()
    out_d = nc.dram_tensor("out", [N, D], F32, kind="ExternalOutput").ap()

    def bcast(ap_1xN, parts=128):
        # [1, n] -> [parts, n] partition-broadcast read (DMA only)
        return bass.AP(
            tensor=ap_1xN.tensor, offset=ap_1xN.offset, ap=[[0, parts]] + ap_1xN.ap[1:]
        )

    with (
        tc.tile_pool(name="const", bufs=1) as const,
        tc.tile_pool(name="xin", bufs=2) as xin,
        tc.tile_pool(name="ln", bufs=3) as ln,
        tc.tile_pool(name="xnbfp", bufs=2) as xnbfp,
        tc.tile_pool(name="wqs", bufs=2) as wqs,
        tc.tile_pool(name="spp", bufs=2) as spp,
        tc.tile_pool(name="q2p", bufs=4) as q2p,
        tc.tile_pool(name="k2p", bufs=4) as k2p,
        tc.tile_pool(name="etp", bufs=20) as etp,
        tc.tile_pool(name="reclp", bufs=2) as reclp,
        tc.tile_pool(name="recbcp", bufs=2) as recbcp,
        tc.tile_pool(name="drsp", bufs=2, space="DRAM") as drsp,
        tc.tile_pool(name="outp", bufs=2) as outp,
        tc.tile_pool(name="psA", bufs=2, space="PSUM") as psA,
        tc.tile_pool(name="psOun", bufs=2, space="PSUM") as psOun,
    ):
        # ---- resident constants -------------------------------------
        wv_sb = const.tile([128, DT_, 1024], BF16, tag="wv")
        nc.gpsimd.dma_start(out=wv_sb[:], in_=wv_d.rearrange("(t p) e -> p t e", p=128))
        wout_sb = const.tile([128, DT_, 1024], BF16, tag="wout")
        nc.gpsimd.dma_start(
            out=wout_sb[:], in_=wout_d.rearrange("(t p) e -> p t e", p=128)
        )
        csq_sb = const.tile([128, N], BF16, tag="csq")
        nc.gpsimd.dma_start(out=csq_sb[:], in_=csq_d)
        csk_sb = const.tile([128, N], BF16, tag="csk")
        nc.gpsimd.dma_start(out=csk_sb[:], in_=csk_d)
        qbias_sb = const.tile([128, 16], F32, tag="qbias")
        nc.gpsimd.dma_start(out=qbias_sb[:], in_=qbias_d)
        vb_sb = const.tile([128, 1024], F32, tag="vb")
        nc.gpsimd.dma_start(out=vb_sb[:], in_=bcast(vbias_d))
        bout_sb = const.tile([128, 1024], F32, tag="bout")
        nc.gpsimd.dma_start(out=bout_sb[:], in_=bcast(bout_d))
        eps_sb = const.tile([128, 1], F32, tag="eps")
        nc.vector.memset(eps_sb[:], 1e-5)
        ident = const.tile([128, 128], F32, tag="ident")
        make_identity(nc, ident[:])

        xnT = const.tile([128, DT_, N], BF16, tag="xnT")
        vp = const.tile([128, MC_, H * 65], BF16, tag="vp")
        nc.gpsimd.memset(vp[:], 1.0)
        ot_sb = const.tile([128, DT_, N], BF16, tag="otsb")

        # ---- Phase A: layernorm + PE transpose ----------------------
        for c in range(NC_):
            x_t = xin.tile([128, D], F32, tag="x")
            nc.sync.dma_start(out=x_t[:], in_=x_d[c * 128 : (c + 1) * 128, :])
            st = ln.tile([128, 2, 6], F32, tag="st")
            for s in range(2):
                nc.vector.bn_stats(out=st[:, s, :], in_=x_t[:, s * 512 : (s + 1) * 512])
            mv = ln.tile([128, 2], F32, tag="mv")
            nc.vector.bn_aggr(out=mv[:], in_=st[:])
            # rsig = 1/sqrt(var+eps); Sqrt batches on one ACT table here,
            # reciprocal on [128,1] is cheap on DVE (per-partition scalars).
            rsig = ln.tile([128, 1], F32, tag="rsig")
            nc.scalar.activation(rsig[:], mv[:, 1:2], AF.Sqrt, bias=eps_sb[:])
            nc.vector.reciprocal(out=rsig[:], in_=rsig[:])
            xnbf = xnbfp.tile([128, D], F32, tag="xnbf")
            nc.vector.tensor_scalar(
                out=xnbf[:],
                in0=x_t[:],
                scalar1=mv[:, 0:1],
                scalar2=rsig[:],
                op0=ALU.subtract,
                op1=ALU.mult,
            )
            # transpose each 128x128 block as a regular matmul against an
            # identity rhs: out = xnbf_blk.T @ I.  (is_transpose f32 hits a
            # walrus codegen bug on HW.)
            if int(os.environ.get("KERNEL_DBG", "0")) == 4:
                nc.sync.dma_start(
                    out=out_d[c * 128 : (c + 1) * 128, :], in_=xnbf[:]
                )
            pst = psA.tile([128, N], F32, tag="ps")
            for t in range(DT_):
                nc.tensor.matmul(
                    pst[:, t * 128 : (t + 1) * 128],
                    lhsT=xnbf[:, t * 128 : (t + 1) * 128],
                    rhs=ident[:],
                    start=True,
                    stop=True,
                )
            nc.vector.tensor_copy(
                out=xnT[:, :, c * 128 : (c + 1) * 128],
                in_=pst.rearrange("p (t n) -> p t n", n=128),
            )

        dbg = int(os.environ.get("KERNEL_DBG", "0"))
        if dbg == 1:  # dump xnT (converted to f32) and stop
            for t in range(DT_):
                o_t = outp.tile([128, D], F32, tag="of")
                nc.vector.tensor_copy(out=o_t[:], in_=xnT[:, t, :])
                nc.sync.dma_start(
                    out=out_d[t * 128 : (t + 1) * 128, :], in_=o_t[:]
                )
            return

        # ---- helpers ------------------------------------------------
        def qk_pair(j):
            """e-chunk j of q AND k -> q2t/k2t for heads 2j, 2j+1.

            ACT ops are batched [Exp, Exp, Ln, Ln] so the activation table
            switches twice per j-step instead of four times.
            """
            psqk = []
            for is_q in (True, False):
                ecol = j * 128 if is_q else 1024 + j * 128
                wt = wqs.tile([128, DT_, 128], BF16, tag="wt")
                nc.sync.dma_start(
                    out=wt[:],
                    in_=wqk_d.rearrange("(t p) e -> p t e", p=128)[
                        :, :, ecol : ecol + 128
                    ],
                )
                ps = psA.tile([128, N], F32, tag="ps")
                for t in range(DT_):
                    for hlf in range(2):
                        nc.tensor.matmul(
                            ps[:, hlf * 512 : (hlf + 1) * 512],
                            lhsT=wt[:, t, :],
                            rhs=xnT[:, t, hlf * 512 : (hlf + 1) * 512],
                            start=(t == 0),
                            stop=(t == DT_ - 1),
                        )
                psqk.append(ps)
            # softplus(x + qb) = ln(1 + exp(x + qb)); exp runs in-place on
            # the PSUM tile, Ln(.+1) drains PSUM -> SBUF bf16.  Safe here:
            # |x + qb| <~ 8 for this problem's data, so no exp overflow.
            for is_q, ps in zip((True, False), psqk):
                bcol = j if is_q else 8 + j
                nc.scalar.activation(
                    ps[:], ps[:], AF.Exp, bias=qbias_sb[:, bcol : bcol + 1]
                )
            sps = []
            for ps in psqk:
                sp = spp.tile([128, N], BF16, tag="sp")
                nc.scalar.activation(sp[:], ps[:], AF.Ln, bias=1.0)
                sps.append(sp)
            out = []
            for is_q, sp in zip((True, False), sps):
                pool = q2p if is_q else k2p
                cs = csq_sb if is_q else csk_sb
                tiles = []
                for hh in range(2):  # head 2j+hh
                    dup = pool.tile([128, N], BF16, tag="d")
                    nc.sync.dma_start(
                        out=dup[0:64, :], in_=sp[hh * 64 : hh * 64 + 64, :]
                    )
                    nc.sync.dma_start(
                        out=dup[64:128, :], in_=sp[hh * 64 : hh * 64 + 64, :]
                    )
                    nc.vector.tensor_mul(out=dup[:], in0=dup[:], in1=cs[:])
                    tiles.append(dup)
                out.append(tiles)
            return out  # [q_tiles, k_tiles]

        et_tiles = {}

        def dots(h, q2, k2):
            ets = []
            for i in range(MC_):
                ps = psA.tile([128, N], F32, tag="ps")
                for hlf in range(2):
                    nc.tensor.matmul(
                        ps[:, hlf * 512 : (hlf + 1) * 512],
                        lhsT=k2[:, i * 128 : (i + 1) * 128],
                        rhs=q2[:, hlf * 512 : (hlf + 1) * 512],
                        start=True,
                        stop=True,
                    )
                et = etp.tile([128, N], BF16, tag="et")
                nc.scalar.activation(et[:], ps[:], AF.Exp, scale=SCALE)
                ets.append(et)
            et_tiles[h] = ets

        def stage2(h):
            """O_un^T = V'.T @ E^T; normalize; write O^T head rows."""
            ets = et_tiles.pop(h)
            even = h % 2 == 0
            po = psOun.tile([128, N], F32, tag="oun")
            for i in range(MC_):
                for hlf in range(2):
                    nc.tensor.matmul(
                        po[0:65, hlf * 512 : (hlf + 1) * 512],
                        lhsT=vp[:, i, h * 65 : (h + 1) * 65],
                        rhs=ets[i][:, hlf * 512 : (hlf + 1) * 512],
                        start=(i == 0),
                        stop=(i == MC_ - 1),
                    )
            # Drain PSUM to SBUF immediately (frees the psum slot for the
            # next head).  The [1,1024] L-row reciprocal would cost
            # free-size*8cyc = 6.5us on DVE and block its queue, so fold
            # the row into a [128, 8] column block via DRAM-bounce DMAs
            # (recip then costs ~0.1us), bounce back, and
            # partition-broadcast from DRAM (SBUF sources can't use
            # step-0 partition APs).
            po_sb = reclp.tile([128, N], F32, tag="posb")
            nc.vector.tensor_copy(out=po_sb[0:65, :], in_=po[0:65, :])
            drs = drsp.tile([1, N], F32, tag="drs")
            nc.sync.dma_start(out=drs[:], in_=po_sb[64:65, :])
            lcol = reclp.tile([128, 8], F32, tag="lcol")
            nc.sync.dma_start(
                out=lcol[:], in_=drs.rearrange("o (c p) -> (o p) c", p=128)
            )
            lcolr = reclp.tile([128, 8], F32, tag="lcolr")
            nc.vector.reciprocal(out=lcolr[:], in_=lcol[:])
            drs2 = drsp.tile([1, N], F32, tag="drs2")
            nc.sync.dma_start(
                out=drs2.rearrange("o (c p) -> (o p) c", p=128), in_=lcolr[:]
            )
            lbc = recbcp.tile([128, N], F32, tag="lbc")
            nc.sync.dma_start(out=lbc[0:64, :], in_=bcast(drs2[0:1, :], 64))
            if even:
                nc.vector.tensor_mul(
                    out=ot_sb[0:64, h // 2, :],
                    in0=po_sb[0:64, :],
                    in1=lbc[0:64, :],
                )
            else:
                # DVE can't shift partitions; bounce through SBUF + DMA.
                otmp = reclp.tile([128, N], BF16, tag="otmp")
                nc.vector.tensor_mul(
                    out=otmp[0:64, :], in0=po_sb[0:64, :], in1=lbc[0:64, :]
                )
                nc.sync.dma_start(
                    out=ot_sb[64:128, h // 2, :], in_=otmp[0:64, :]
                )

        # ---- Phases B/C/D interleaved -------------------------------
        q0, k0 = qk_pair(0)
        dots(0, q0[0], k0[0])
        dots(1, q0[1], k0[1])

        for c in range(NC_):  # Phase B: v projection (fills ACT slack)
            ps = psA.tile([128, N], F32, tag="ps")
            for t in range(DT_):
                for hlf in range(2):
                    nc.tensor.matmul(
                        ps[:, hlf * 512 : (hlf + 1) * 512],
                        lhsT=xnT[:, t, c * 128 : (c + 1) * 128],
                        rhs=wv_sb[:, t, hlf * 512 : (hlf + 1) * 512],
                        start=(t == 0),
                        stop=(t == DT_ - 1),
                    )
            # V' layout: [v(64) | 1] per head
            vpr = vp[:, c, :].rearrange("p (h w) -> p h w", w=65)
            nc.vector.tensor_add(
                out=vpr[:, :, 0:64],
                in0=ps.rearrange("p (h w) -> p h w", w=64),
                in1=vb_sb.rearrange("p (h w) -> p h w", w=64),
            )

        for j in range(1, EC_Q):
            qj, kj = qk_pair(j)
            dots(2 * j, qj[0], kj[0])
            stage2(2 * j - 2)
            dots(2 * j + 1, qj[1], kj[1])
            stage2(2 * j - 1)
        stage2(14)
        stage2(15)

        # ---- Phase F: output projection -----------------------------
        for c in range(NC_):
            ps = psA.tile([128, N], F32, tag="ps")
            for t in range(DT_):
                for hlf in range(2):
                    nc.tensor.matmul(
                        ps[:, hlf * 512 : (hlf + 1) * 512],
                        lhsT=ot_sb[:, t, c * 128 : (c + 1) * 128],
                        rhs=wout_sb[:, t, hlf * 512 : (hlf + 1) * 512],
                        start=(t == 0),
                        stop=(t == DT_ - 1),
                    )
            o_t = outp.tile([128, D], F32, tag="of")
            nc.vector.tensor_add(out=o_t[:], in0=ps[:], in1=bout_sb[:])
            nc.sync.dma_start(out=out_d[c * 128 : (c + 1) * 128, :], in_=o_t[:])


_NC_CACHE = {}


def _get_nc():
    if "nc" not in _NC_CACHE:
        # NOTE: do NOT reorder activation tables to pin
        # natural_log_exp_and_others — that table has no valid runtime
        # payload on HW (all-NaN results).  Instead the emit order batches
        # same-function ACT ops to minimize table switches.
        nc = bacc.Bacc(
            "TRN2",
            target_bir_lowering=False,
            debug=False,
            enable_asserts=False,
            num_devices=8,
        )
        with tile.TileContext(nc) as tc:
            _emit(tc)
        nc.compile()
        _NC_CACHE["nc"] = nc
    return _NC_CACHE["nc"]


def _trace_ok():
    try:
        from antenv.axon_hooks import get_axon_ntff_profile_hook

        return get_axon_ntff_profile_hook() is not None
    except Exception:
        return False


def kernel(**inputs):
    bf = ml_dtypes.bfloat16
    x = np.ascontiguousarray(np.asarray(inputs["x"], dtype=np.float32))
    freqs = np.asarray(inputs["freqs"], dtype=np.float32)[0]
    fbias = np.asarray(inputs["bias"], dtype=np.float32)[0]
    g = np.asarray(inputs["ln_gamma"], dtype=np.float32)
    be = np.asarray(inputs["ln_beta"], dtype=np.float32)
    w_qk = np.asarray(inputs["w_qk"], dtype=np.float32)
    w_v = np.asarray(inputs["w_v"], dtype=np.float32)
    w_out = np.asarray(inputs["w_out"], dtype=np.float32)
    b_out = np.asarray(inputs["b_out"], dtype=np.float32)

    wqk_s = np.ascontiguousarray((w_qk * g[:, None]).astype(bf))
    wv_s = np.ascontiguousarray((w_v * g[:, None]).astype(bf))
    wout_b = np.ascontiguousarray(w_out.astype(bf))
    qb = be @ w_qk  # [2048]; beta contribution to q/k pre-activation
    vb = (be @ w_v).astype(np.float32)[None, :]
    qbias = np.ascontiguousarray(qb.reshape(16, 128).T.astype(np.float32))
    csq = np.ascontiguousarray(
        np.concatenate([np.cos(freqs).T, np.sin(freqs).T], axis=0).astype(bf)
    )
    fb = freqs + fbias
    csk = np.ascontiguousarray(
        np.concatenate([np.cos(fb).T, np.sin(fb).T], axis=0).astype(bf)
    )
    bout = np.ascontiguousarray(b_out[None, :].astype(np.float32))

    shared = dict(
        wqk=wqk_s, wv=wv_s, wout=wout_b, csq=csq, csk=csk,
        qbias=qbias, vbias=vb, bout=bout,
    )
    in_maps = [dict(x=np.ascontiguousarray(x[i]), **shared) for i in range(B)]

    nc = _get_nc()
    want_trace = bool(int(os.environ.get("KERNEL_TRACE", "0")))
    res = run_bass_kernel_spmd(
        nc,
        in_maps,
        core_ids=list(range(B)),
        trace=want_trace and _trace_ok(),
    )
    out = np.stack([res.results[i]["out"] for i in range(B)], axis=0)
    if getattr(res, "exec_time_ns", None):
        kernel.last_exec_time_ns = res.exec_time_ns
    kernel.last_results = res
    return out


# revision 28
# speedup vs baseline: 1.0460x; 1.0460x over previous
"""Fused attention kernel for TRN2, data-parallel over 8 NeuronCores.

Problem: LN -> qk/v projections -> softplus-polar embedding -> attention
-> output projection.  B=8 batch elements are sharded one-per-core; each
core runs an identical single-core program (no collectives).

Layout strategy (per core, N=1024 tokens, D=1024, H=16 heads, DH=64):
  - LN in [n, d] layout (free-axis reductions), then PE-transpose to
    xnT [d, n] (bf16) for the projection GEMMs.
  - q/k produced TRANSPOSED ([e, n]) with w_qk tiles as the stationary
    operand; v produced natural ([n, e]) with xnT as stationary.
  - polar: softplus = Ln(1+Exp(x)) on ScalarE (Exp in-place on PSUM; the
    Ln/Exp pair lives in ONE activation table set, no table thrash), the
    64-row head slice is duplicated into both partition halves by two
    SBUF->SBUF DMAs, then one DVE multiply against a [cosT; sinT] table.
  - attention entirely in transposed layout: S^T = k2t.T @ q2t (K=128,
    single k-tile), exp via ScalarE (scale=DH^-0.5 fused), then
    O_un^T = V'.T @ E^T with V' stationary [128, 65] per (m-tile, head).
    V' carries a ones-column (last col for even heads, FIRST col for odd
    heads) so the softmax denominator L lands on psum row 64/63; O^T for
    a head pair fills one [128, n] e-tile directly -- no O transpose.
    Normalization: DVE reciprocal of the L row, partition-broadcast via
    a strided SBUF->SBUF DMA, one DVE multiply into ot_sb.
  - final GEMM: lhsT = ot tiles, rhs = w_out.

ln_gamma is folded into w_qk/w_v rows on the host; ln_beta enters as an
ACT bias ([128,1] per e-chunk) for q/k and a broadcast add for v; b_out
is a broadcast add on the output.  All exact algebra, ~zero device cost.
"""

import os

import ml_dtypes
import numpy as np

import concourse.bass as bass
import concourse.tile as tile
from concourse import bacc, mybir
from concourse.bass_utils import run_bass_kernel_spmd
from concourse.masks import make_identity

F32 = mybir.dt.float32
BF16 = mybir.dt.bfloat16
AF = mybir.ActivationFunctionType
ALU = mybir.AluOpType

B, N, D, H, DH = 8, 1024, 1024, 16, 64
NC_, DT_, EC_Q, MC_ = 8, 8, 8, 8  # n-chunks, d-tiles, q e-chunks, m-tiles
SCALE = DH ** -0.5


def _emit(tc):
    nc = tc.nc

    x_d = nc.dram_tensor("x", [N, D], F32, kind="ExternalInput").ap()
    wqk_d = nc.dram_tensor("wqk", [D, 2 * H * DH], BF16, kind="ExternalInput").ap()
    wv_d = nc.dram_tensor("wv", [D, H * DH], BF16, kind="ExternalInput").ap()
    wout_d = nc.dram_tensor("wout", [H * DH, D], BF16, kind="ExternalInput").ap()
    csq_d = nc.dram_tensor("csq", [128, N], BF16, kind="ExternalInput").ap()
    csk_d = nc.dram_tensor("csk", [128, N], BF16, kind="ExternalInput").ap()
    qbias_d = nc.dram_tensor("qbias", [128, 16], F32, kind="ExternalInput").ap()
    vbias_d = nc.dram_tensor("vbias", [1, H * DH], F32, kind="ExternalInput").ap()
    bout_d = nc.dram_tensor("bout", [1, D], F32, kind="ExternalInput").ap()
    out_d = nc.dram_tensor("out", [N, D], F32, kind="ExternalOutput").ap()

    def bcast(ap_1xN, parts=128):
        # [1, n] -> [parts, n] partition-broadcast read (DMA only)
        return bass.AP(
            tensor=ap_1xN.tensor, offset=ap_1xN.offset, ap=[[0, parts]] + ap_1xN.ap[1:]
        )

    with (
        tc.tile_pool(name="const", bufs=1) as const,
        tc.tile_pool(name="xin", bufs=2) as xin,
        tc.tile_pool(name="ln", bufs=3) as ln,
        tc.tile_pool(name="xnbfp", bufs=2) as xnbfp,
        tc.tile_pool(name="wqs", bufs=2) as wqs,
        tc.tile_pool(name="spp", bufs=2) as spp,
        tc.tile_pool(name="q2p", bufs=3) as q2p,
        tc.tile_pool(name="k2p", bufs=3) as k2p,
        tc.tile_pool(name="etp", bufs=19) as etp,
        tc.tile_pool(name="reclp", bufs=2) as reclp,
        tc.tile_pool(name="posbp", bufs=4) as posbp,
        tc.tile_pool(name="recbcp", bufs=4) as recbcp,
        tc.tile_pool(name="drsp", bufs=2, space="DRAM") as drsp,
        tc.tile_pool(name="outp", bufs=2) as outp,
        tc.tile_pool(name="psA", bufs=2, space="PSUM") as psA,
        tc.tile_pool(name="psOun", bufs=2, space="PSUM") as psOun,
    ):
        # ---- resident constants -------------------------------------
        wv_sb = const.tile([128, DT_, 1024], BF16, tag="wv")
        nc.gpsimd.dma_start(out=wv_sb[:], in_=wv_d.rearrange("(t p) e -> p t e", p=128))
        wout_sb = const.tile([128, DT_, 1024], BF16, tag="wout")
        nc.gpsimd.dma_start(
            out=wout_sb[:], in_=wout_d.rearrange("(t p) e -> p t e", p=128)
        )
        csq_sb = const.tile([128, N], BF16, tag="csq")
        nc.gpsimd.dma_start(out=csq_sb[:], in_=csq_d)
        csk_sb = const.tile([128, N], BF16, tag="csk")
        nc.gpsimd.dma_start(out=csk_sb[:], in_=csk_d)
        qbias_sb = const.tile([128, 16], F32, tag="qbias")
        nc.gpsimd.dma_start(out=qbias_sb[:], in_=qbias_d)
        vb_sb = const.tile([128, 1024], BF16, tag="vb")
        nc.gpsimd.dma_start(out=vb_sb[:], in_=bcast(vbias_d))
        bout_sb = const.tile([128, 1024], BF16, tag="bout")
        nc.gpsimd.dma_start(out=bout_sb[:], in_=bcast(bout_d))
        eps_sb = const.tile([128, 1], F32, tag="eps")
        nc.vector.memset(eps_sb[:], 1e-5)
        ident = const.tile([128, 128], F32, tag="ident")
        make_identity(nc, ident[:])

        xnT = const.tile([128, DT_, N], BF16, tag="xnT")
        vp = const.tile([128, MC_, H * 65], BF16, tag="vp")
        nc.gpsimd.memset(vp[:], 1.0)
        ot_sb = const.tile([128, DT_, N], BF16, tag="otsb")

        # ---- Phase A: layernorm + PE transpose ----------------------
        for c in range(NC_):
            x_t = xin.tile([128, D], F32, tag="x")
            nc.sync.dma_start(out=x_t[:], in_=x_d[c * 128 : (c + 1) * 128, :])
            st = ln.tile([128, 2, 6], F32, tag="st")
            for s in range(2):
                nc.vector.bn_stats(out=st[:, s, :], in_=x_t[:, s * 512 : (s + 1) * 512])
            mv = ln.tile([128, 2], F32, tag="mv")
            nc.vector.bn_aggr(out=mv[:], in_=st[:])
            # rsig = 1/sqrt(var+eps); Sqrt batches on one ACT table here,
            # reciprocal on [128,1] is cheap on DVE (per-partition scalars).
            rsig = ln.tile([128, 1], F32, tag="rsig")
            nc.scalar.activation(rsig[:], mv[:, 1:2], AF.Sqrt, bias=eps_sb[:])
            nc.vector.reciprocal(out=rsig[:], in_=rsig[:])
            xnbf = xnbfp.tile([128, D], F32, tag="xnbf")
            nc.vector.tensor_scalar(
                out=xnbf[:],
                in0=x_t[:],
                scalar1=mv[:, 0:1],
                scalar2=rsig[:],
                op0=ALU.subtract,
                op1=ALU.mult,
            )
            # transpose each 128x128 block as a regular matmul against an
            # identity rhs: out = xnbf_blk.T @ I.  (is_transpose f32 hits a
            # walrus codegen bug on HW.)
            if int(os.environ.get("KERNEL_DBG", "0")) == 4:
                nc.sync.dma_start(
                    out=out_d[c * 128 : (c + 1) * 128, :], in_=xnbf[:]
                )
            pst = psA.tile([128, N], F32, tag="ps")
            for t in range(DT_):
                nc.tensor.matmul(
                    pst[:, t * 128 : (t + 1) * 128],
                    lhsT=xnbf[:, t * 128 : (t + 1) * 128],
                    rhs=ident[:],
                    start=True,
                    stop=True,
                )
            nc.vector.tensor_copy(
                out=xnT[:, :, c * 128 : (c + 1) * 128],
                in_=pst.rearrange("p (t n) -> p t n", n=128),
            )

        dbg = int(os.environ.get("KERNEL_DBG", "0"))
        if dbg == 1:  # dump xnT (converted to f32) and stop
            for t in range(DT_):
                o_t = outp.tile([128, D], F32, tag="of")
                nc.vector.tensor_copy(out=o_t[:], in_=xnT[:, t, :])
                nc.sync.dma_start(
                    out=out_d[t * 128 : (t + 1) * 128, :], in_=o_t[:]
                )
            return

        # ---- helpers ------------------------------------------------
        def qk_pair(j):
            """e-chunk j of q AND k -> q2t/k2t for heads 2j, 2j+1.

            ACT ops are batched [Exp, Exp, Ln, Ln] so the activation table
            switches twice per j-step instead of four times.
            """
            psqk = []
            for is_q in (True, False):
                ecol = j * 128 if is_q else 1024 + j * 128
                wt = wqs.tile([128, DT_, 128], BF16, tag="wt")
                nc.sync.dma_start(
                    out=wt[:],
                    in_=wqk_d.rearrange("(t p) e -> p t e", p=128)[
                        :, :, ecol : ecol + 128
                    ],
                )
                ps = psA.tile([128, N], F32, tag="ps")
                for t in range(DT_):
                    for hlf in range(2):
                        nc.tensor.matmul(
                            ps[:, hlf * 512 : (hlf + 1) * 512],
                            lhsT=wt[:, t, :],
                            rhs=xnT[:, t, hlf * 512 : (hlf + 1) * 512],
                            start=(t == 0),
                            stop=(t == DT_ - 1),
                        )
                psqk.append(ps)
            # softplus(x + qb) = ln(1 + exp(x + qb)); exp runs in-place on
            # the PSUM tile, Ln(.+1) drains PSUM -> SBUF bf16.  Safe here:
            # |x + qb| <~ 8 for this problem's data, so no exp overflow.
            for is_q, ps in zip((True, False), psqk):
                bcol = j if is_q else 8 + j
                nc.scalar.activation(
                    ps[:], ps[:], AF.Exp, bias=qbias_sb[:, bcol : bcol + 1]
                )
            sps = []
            for ps in psqk:
                sp = spp.tile([128, N], BF16, tag="sp")
                nc.scalar.activation(sp[:], ps[:], AF.Ln, bias=1.0)
                sps.append(sp)
            out = []
            for is_q, sp in zip((True, False), sps):
                pool = q2p if is_q else k2p
                cs = csq_sb if is_q else csk_sb
                tiles = []
                for hh in range(2):  # head 2j+hh
                    dup = pool.tile([128, N], BF16, tag="d")
                    nc.sync.dma_start(
                        out=dup[0:64, :], in_=sp[hh * 64 : hh * 64 + 64, :]
                    )
                    nc.sync.dma_start(
                        out=dup[64:128, :], in_=sp[hh * 64 : hh * 64 + 64, :]
                    )
                    nc.vector.tensor_mul(out=dup[:], in0=dup[:], in1=cs[:])
                    tiles.append(dup)
                out.append(tiles)
            return out  # [q_tiles, k_tiles]

        et_tiles = {}

        def dots(h, q2, k2):
            ets = []
            for i in range(MC_):
                ps = psA.tile([128, N], F32, tag="ps")
                for hlf in range(2):
                    nc.tensor.matmul(
                        ps[:, hlf * 512 : (hlf + 1) * 512],
                        lhsT=k2[:, i * 128 : (i + 1) * 128],
                        rhs=q2[:, hlf * 512 : (hlf + 1) * 512],
                        start=True,
                        stop=True,
                    )
                et = etp.tile([128, N], BF16, tag="et")
                nc.scalar.activation(et[:], ps[:], AF.Exp, scale=SCALE)
                ets.append(et)
            et_tiles[h] = ets

        fin_state = {}

        def stage2_mm(h):
            """O_un^T = V'.T @ E^T; drain PSUM; kick off the 1/L chain."""
            ets = et_tiles.pop(h)
            po = psOun.tile([128, N], F32, tag="oun")
            for i in range(MC_):
                for hlf in range(2):
                    nc.tensor.matmul(
                        po[0:65, hlf * 512 : (hlf + 1) * 512],
                        lhsT=vp[:, i, h * 65 : (h + 1) * 65],
                        rhs=ets[i][:, hlf * 512 : (hlf + 1) * 512],
                        start=(i == 0),
                        stop=(i == MC_ - 1),
                    )
            # Drain PSUM to SBUF immediately (frees the psum slot for the
            # next head).  The [1,1024] L-row reciprocal would cost
            # free-size*8cyc = 6.5us on DVE and block its queue, so fold
            # the row into a [128, 8] column block via DRAM-bounce DMAs
            # (recip then costs ~0.1us), bounce back, and
            # partition-broadcast from DRAM (SBUF sources can't use
            # step-0 partition APs).
            po_sb = posbp.tile([128, N], F32, tag="posb")
            nc.vector.tensor_copy(out=po_sb[0:65, :], in_=po[0:65, :])
            drs = drsp.tile([1, N], F32, tag="drs")
            nc.sync.dma_start(out=drs[:], in_=po_sb[64:65, :])
            lcol = reclp.tile([128, 8], F32, tag="lcol")
            nc.sync.dma_start(
                out=lcol[:], in_=drs.rearrange("o (c p) -> (o p) c", p=128)
            )
            lcolr = reclp.tile([128, 8], F32, tag="lcolr")
            nc.vector.reciprocal(out=lcolr[:], in_=lcol[:])
            drs2 = drsp.tile([1, N], F32, tag="drs2")
            nc.sync.dma_start(
                out=drs2.rearrange("o (c p) -> (o p) c", p=128), in_=lcolr[:]
            )
            lbc = recbcp.tile([128, N], F32, tag="lbc")
            nc.sync.dma_start(out=lbc[0:64, :], in_=bcast(drs2[0:1, :], 64))
            fin_state[h] = (po_sb, lbc)

        def stage2_fin(h):
            """One j-step later: multiply O_un by the broadcast 1/L."""
            po_sb, lbc = fin_state.pop(h)
            if h % 2 == 0:
                nc.vector.tensor_mul(
                    out=ot_sb[0:64, h // 2, :],
                    in0=po_sb[0:64, :],
                    in1=lbc[0:64, :],
                )
            else:
                # DVE can't shift partitions; bounce through SBUF + DMA.
                otmp = reclp.tile([128, N], BF16, tag="otmp")
                nc.vector.tensor_mul(
                    out=otmp[0:64, :], in0=po_sb[0:64, :], in1=lbc[0:64, :]
                )
                nc.sync.dma_start(
                    out=ot_sb[64:128, h // 2, :], in_=otmp[0:64, :]
                )

        # ---- Phases B/C/D interleaved -------------------------------
        q0, k0 = qk_pair(0)
        dots(0, q0[0], k0[0])
        dots(1, q0[1], k0[1])

        for c in range(NC_):  # Phase B: v projection (fills ACT slack)
            ps = psA.tile([128, N], F32, tag="ps")
            for t in range(DT_):
                for hlf in range(2):
                    nc.tensor.matmul(
                        ps[:, hlf * 512 : (hlf + 1) * 512],
                        lhsT=xnT[:, t, c * 128 : (c + 1) * 128],
                        rhs=wv_sb[:, t, hlf * 512 : (hlf + 1) * 512],
                        start=(t == 0),
                        stop=(t == DT_ - 1),
                    )
            # V' layout: [v(64) | 1] per head
            vpr = vp[:, c, :].rearrange("p (h w) -> p h w", w=65)
            nc.vector.tensor_add(
                out=vpr[:, :, 0:64],
                in0=ps.rearrange("p (h w) -> p h w", w=64),
                in1=vb_sb.rearrange("p (h w) -> p h w", w=64),
            )

        for j in range(1, EC_Q):
            qj, kj = qk_pair(j)
            dots(2 * j, qj[0], kj[0])
            stage2_mm(2 * j - 2)
            dots(2 * j + 1, qj[1], kj[1])
            stage2_mm(2 * j - 1)
            if j >= 2:
                stage2_fin(2 * j - 4)
                stage2_fin(2 * j - 3)
        stage2_mm(14)
        stage2_mm(15)
        for h in (12, 13, 14, 15):
            stage2_fin(h)

        # ---- Phase F: output projection -----------------------------
        for c in range(NC_):
            ps = psA.tile([128, N], F32, tag="ps")
            for t in range(DT_):
                for hlf in range(2):
                    nc.tensor.matmul(
                        ps[:, hlf * 512 : (hlf + 1) * 512],
                        lhsT=ot_sb[:, t, c * 128 : (c + 1) * 128],
                        rhs=wout_sb[:, t, hlf * 512 : (hlf + 1) * 512],
                        start=(t == 0),
                        stop=(t == DT_ - 1),
                    )
            o_t = outp.tile([128, D], F32, tag="of")
            nc.vector.tensor_add(out=o_t[:], in0=ps[:], in1=bout_sb[:])
            nc.sync.dma_start(out=out_d[c * 128 : (c + 1) * 128, :], in_=o_t[:])


_NC_CACHE = {}


def _get_nc():
    if "nc" not in _NC_CACHE:
        # NOTE: do NOT reorder activation tables to pin
        # natural_log_exp_and_others — that table has no valid runtime
        # payload on HW (all-NaN results).  Instead the emit order batches
        # same-function ACT ops to minimize table switches.
        nc = bacc.Bacc(
            "TRN2",
            target_bir_lowering=False,
            debug=False,
            enable_asserts=False,
            num_devices=8,
        )
        with tile.TileContext(nc) as tc:
            _emit(tc)
        nc.compile()
        _NC_CACHE["nc"] = nc
    return _NC_CACHE["nc"]


def _trace_ok():
    try:
        from antenv.axon_hooks import get_axon_ntff_profile_hook

        return get_axon_ntff_profile_hook() is not None
    except Exception:
        return False


def kernel(**inputs):
    bf = ml_dtypes.bfloat16
    x = np.ascontiguousarray(np.asarray(inputs["x"], dtype=np.float32))
    freqs = np.asarray(inputs["freqs"], dtype=np.float32)[0]
    fbias = np.asarray(inputs["bias"], dtype=np.float32)[0]
    g = np.asarray(inputs["ln_gamma"], dtype=np.float32)
    be = np.asarray(inputs["ln_beta"], dtype=np.float32)
    w_qk = np.asarray(inputs["w_qk"], dtype=np.float32)
    w_v = np.asarray(inputs["w_v"], dtype=np.float32)
    w_out = np.asarray(inputs["w_out"], dtype=np.float32)
    b_out = np.asarray(inputs["b_out"], dtype=np.float32)

    wqk_s = np.ascontiguousarray((w_qk * g[:, None]).astype(bf))
    wv_s = np.ascontiguousarray((w_v * g[:, None]).astype(bf))
    wout_b = np.ascontiguousarray(w_out.astype(bf))
    qb = be @ w_qk  # [2048]; beta contribution to q/k pre-activation
    vb = (be @ w_v).astype(np.float32)[None, :]
    qbias = np.ascontiguousarray(qb.reshape(16, 128).T.astype(np.float32))
    csq = np.ascontiguousarray(
        np.concatenate([np.cos(freqs).T, np.sin(freqs).T], axis=0).astype(bf)
    )
    fb = freqs + fbias
    csk = np.ascontiguousarray(
        np.concatenate([np.cos(fb).T, np.sin(fb).T], axis=0).astype(bf)
    )
    bout = np.ascontiguousarray(b_out[None, :].astype(np.float32))

    shared = dict(
        wqk=wqk_s, wv=wv_s, wout=wout_b, csq=csq, csk=csk,
        qbias=qbias, vbias=vb, bout=bout,
    )
    in_maps = [dict(x=np.ascontiguousarray(x[i]), **shared) for i in range(B)]

    nc = _get_nc()
    want_trace = bool(int(os.environ.get("KERNEL_TRACE", "0")))
    res = run_bass_kernel_spmd(
        nc,
        in_maps,
        core_ids=list(range(B)),
        trace=want_trace and _trace_ok(),
    )
    out = np.stack([res.results[i]["out"] for i in range(B)], axis=0)
    if getattr(res, "exec_time_ns", None):
        kernel.last_exec_time_ns = res.exec_time_ns
    kernel.last_results = res
    return out


# revision 29
# speedup vs baseline: 1.1018x; 1.0534x over previous
"""Fused attention kernel for TRN2, data-parallel over 8 NeuronCores.

Problem: LN -> qk/v projections -> softplus-polar embedding -> attention
-> output projection.  B=8 batch elements are sharded one-per-core; each
core runs an identical single-core program (no collectives).

Layout strategy (per core, N=1024 tokens, D=1024, H=16 heads, DH=64):
  - LN in [n, d] layout (free-axis reductions), PE-transpose (matmul
    against an identity rhs) to xnT [d, n] bf16; the v projection for
    each n-chunk is interleaved so PE works while DVE does LN stats.
  - q/k produced TRANSPOSED ([e, n]) with w_qk tiles stationary.
  - polar: softplus = Ln(1+Exp(x)) on ScalarE (Exp in-place on PSUM; ACT
    ops batched [Exp,Exp,Ln,Ln] to limit activation-table switches), the
    64-row head slice is duplicated into both partition halves by two
    SBUF->SBUF DMAs, then one DVE multiply against a [cosT; sinT] table.
  - attention in transposed layout: S^T = k2t.T @ q2t (K=128, one
    k-tile), exp on ScalarE (scale=DH^-0.5 fused) straight from PSUM,
    O_un^T = V'.T @ E^T with V' [m,65] stationary (ones-column ->
    softmax denominator L on psum row 64); a head pair fills one [128,n]
    e-tile of O^T directly - no O transpose.
  - 1/L: the [1,1024] L row would cost free_size*8cyc on DVE, so it is
    folded to a [128,8] column block via DRAM-bounce DMAs, recip'd
    cheaply, bounced back, and partition-broadcast from DRAM.
  - the whole attention middle runs as a lag-2 software pipeline:
    qk_pair(j+2) is produced during step j, so the serial
    PE(qk)->ACT(softplus)->DVE(polar)->PE(dots) chain never stalls PE;
    stage2's normalize multiply lags one step behind its matmuls so the
    1/L DMA chain latency is hidden.
  - final GEMM: lhsT = O^T tiles, rhs = w_out.

ln_gamma is folded into w_qk/w_v rows on the host; ln_beta enters as an
ACT bias ([128,1] per e-chunk) for q/k and a broadcast add for v; b_out
is a broadcast add on the output.  All exact algebra, ~zero device cost.
"""

import os

import ml_dtypes
import numpy as np

import concourse.bass as bass
import concourse.tile as tile
from concourse import bacc, mybir
from concourse.bass_utils import run_bass_kernel_spmd
from concourse.masks import make_identity

F32 = mybir.dt.float32
BF16 = mybir.dt.bfloat16
AF = mybir.ActivationFunctionType
ALU = mybir.AluOpType

B, N, D, H, DH = 8, 1024, 1024, 16, 64
NC_, DT_, EC_Q, MC_ = 8, 8, 8, 8  # n-chunks, d-tiles, q e-chunks, m-tiles
SCALE = DH ** -0.5


def _emit(tc):
    nc = tc.nc

    x_d = nc.dram_tensor("x", [N, D], F32, kind="ExternalInput").ap()
    wqk_d = nc.dram_tensor("wqk", [D, 2 * H * DH], BF16, kind="ExternalInput").ap()
    wv_d = nc.dram_tensor("wv", [D, H * DH], BF16, kind="ExternalInput").ap()
    wout_d = nc.dram_tensor("wout", [H * DH, D], BF16, kind="ExternalInput").ap()
    csq_d = nc.dram_tensor("csq", [128, N], BF16, kind="ExternalInput").ap()
    csk_d = nc.dram_tensor("csk", [128, N], BF16, kind="ExternalInput").ap()
    qbias_d = nc.dram_tensor("qbias", [128, 16], F32, kind="ExternalInput").ap()
    vbias_d = nc.dram_tensor("vbias", [1, H * DH], F32, kind="ExternalInput").ap()
    bout_d = nc.dram_tensor("bout", [1, D], F32, kind="ExternalInput").ap()
    out_d = nc.dram_tensor("out", [N, D], F32, kind="ExternalOutput").ap()

    def bcast(ap_1xN, parts=128):
        # [1, n] -> [parts, n] partition-broadcast read (DMA, DRAM src only)
        return bass.AP(
            tensor=ap_1xN.tensor, offset=ap_1xN.offset, ap=[[0, parts]] + ap_1xN.ap[1:]
        )

    with (
        tc.tile_pool(name="const", bufs=1) as const,
        tc.tile_pool(name="wqs", bufs=2) as wqs,
        tc.tile_pool(name="spp", bufs=2) as spp,
        tc.tile_pool(name="etp", bufs=18) as etp,
        tc.tile_pool(name="reclp", bufs=2) as reclp,
        tc.tile_pool(name="drsp", bufs=4, space="DRAM") as drsp,
        tc.tile_pool(name="psA", bufs=2, space="PSUM") as psA,
        tc.tile_pool(name="psOun", bufs=2, space="PSUM") as psOun,
    ):
        # ---- resident constants -------------------------------------
        wv_sb = const.tile([128, DT_, 1024], BF16, tag="wv")
        nc.gpsimd.dma_start(out=wv_sb[:], in_=wv_d.rearrange("(t p) e -> p t e", p=128))
        wout_sb = const.tile([128, DT_, 1024], BF16, tag="wout")
        nc.gpsimd.dma_start(
            out=wout_sb[:], in_=wout_d.rearrange("(t p) e -> p t e", p=128)
        )
        csq_sb = const.tile([128, N], BF16, tag="csq")
        nc.gpsimd.dma_start(out=csq_sb[:], in_=csq_d)
        csk_sb = const.tile([128, N], BF16, tag="csk")
        nc.gpsimd.dma_start(out=csk_sb[:], in_=csk_d)
        qbias_sb = const.tile([128, 16], F32, tag="qbias")
        nc.gpsimd.dma_start(out=qbias_sb[:], in_=qbias_d)
        vb_sb = const.tile([128, 1024], BF16, tag="vb")
        nc.gpsimd.dma_start(out=vb_sb[:], in_=bcast(vbias_d))
        bout_sb = const.tile([128, 1024], BF16, tag="bout")
        nc.gpsimd.dma_start(out=bout_sb[:], in_=bcast(bout_d))
        eps_sb = const.tile([128, 1], F32, tag="eps")
        nc.vector.memset(eps_sb[:], 1e-5)
        ident = const.tile([128, 128], F32, tag="ident")
        make_identity(nc, ident[:])

        xnT = const.tile([128, DT_, N], BF16, tag="xnT")
        vp = const.tile([128, MC_, H * 65], BF16, tag="vp")
        nc.gpsimd.memset(vp[:], 1.0)
        ot_sb = const.tile([128, DT_, N], BF16, tag="otsb")

        # ---- Phase A+B: layernorm + transpose + v projection --------
        # (scoped pools: the attention pools below reuse this SBUF)
        with (
            tc.tile_pool(name="xin", bufs=2) as xin,
            tc.tile_pool(name="ln", bufs=3) as ln,
            tc.tile_pool(name="xnbfp", bufs=2) as xnbfp,
        ):
            for c in range(NC_):
                x_t = xin.tile([128, D], F32, tag="x")
                nc.sync.dma_start(out=x_t[:], in_=x_d[c * 128 : (c + 1) * 128, :])
                st = ln.tile([128, 2, 6], F32, tag="st")
                for s in range(2):
                    nc.vector.bn_stats(
                        out=st[:, s, :], in_=x_t[:, s * 512 : (s + 1) * 512]
                    )
                mv = ln.tile([128, 2], F32, tag="mv")
                nc.vector.bn_aggr(out=mv[:], in_=st[:])
                rsig = ln.tile([128, 1], F32, tag="rsig")
                nc.scalar.activation(rsig[:], mv[:, 1:2], AF.Sqrt, bias=eps_sb[:])
                nc.vector.reciprocal(out=rsig[:], in_=rsig[:])
                xnbf = xnbfp.tile([128, D], F32, tag="xnbf")
                nc.vector.tensor_scalar(
                    out=xnbf[:],
                    in0=x_t[:],
                    scalar1=mv[:, 0:1],
                    scalar2=rsig[:],
                    op0=ALU.subtract,
                    op1=ALU.mult,
                )
                # transpose 128x128 blocks as matmuls vs an identity rhs
                # (is_transpose f32 hits a walrus codegen bug on HW)
                pst = psA.tile([128, N], F32, tag="ps")
                for t in range(DT_):
                    nc.tensor.matmul(
                        pst[:, t * 128 : (t + 1) * 128],
                        lhsT=xnbf[:, t * 128 : (t + 1) * 128],
                        rhs=ident[:],
                        start=True,
                        stop=True,
                    )
                nc.vector.tensor_copy(
                    out=xnT[:, :, c * 128 : (c + 1) * 128],
                    in_=pst.rearrange("p (t n) -> p t n", n=128),
                )
                # v projection for this n-chunk (PE overlaps LN's DVE)
                psv = psA.tile([128, N], F32, tag="ps")
                for t in range(DT_):
                    for hlf in range(2):
                        nc.tensor.matmul(
                            psv[:, hlf * 512 : (hlf + 1) * 512],
                            lhsT=xnT[:, t, c * 128 : (c + 1) * 128],
                            rhs=wv_sb[:, t, hlf * 512 : (hlf + 1) * 512],
                            start=(t == 0),
                            stop=(t == DT_ - 1),
                        )
                vpr = vp[:, c, :].rearrange("p (h w) -> p h w", w=65)
                nc.vector.tensor_add(
                    out=vpr[:, :, 0:64],
                    in0=psv.rearrange("p (h w) -> p h w", w=64),
                    in1=vb_sb.rearrange("p (h w) -> p h w", w=64),
                )

        with (
            tc.tile_pool(name="q2p", bufs=6) as q2p,
            tc.tile_pool(name="k2p", bufs=6) as k2p,
            tc.tile_pool(name="posbp", bufs=4) as posbp,
            tc.tile_pool(name="recbcp", bufs=4) as recbcp,
            tc.tile_pool(name="outp", bufs=2) as outp,
        ):

            def qk_pair(j):
                """e-chunk j of q AND k -> q2t/k2t tiles for heads 2j, 2j+1.

                ACT ops batch [Exp, Exp, Ln, Ln]: two activation-table
                switches per step instead of four.
                """
                psqk = []
                for is_q in (True, False):
                    ecol = j * 128 if is_q else 1024 + j * 128
                    wt = wqs.tile([128, DT_, 128], BF16, tag="wt")
                    nc.sync.dma_start(
                        out=wt[:],
                        in_=wqk_d.rearrange("(t p) e -> p t e", p=128)[
                            :, :, ecol : ecol + 128
                        ],
                    )
                    ps = psA.tile([128, N], F32, tag="ps")
                    for t in range(DT_):
                        for hlf in range(2):
                            nc.tensor.matmul(
                                ps[:, hlf * 512 : (hlf + 1) * 512],
                                lhsT=wt[:, t, :],
                                rhs=xnT[:, t, hlf * 512 : (hlf + 1) * 512],
                                start=(t == 0),
                                stop=(t == DT_ - 1),
                            )
                    psqk.append(ps)
                # softplus(x+qb) = ln(1+exp(x+qb)); exp in-place on PSUM.
                # Safe: |x+qb| <~ 8 for this problem's data.
                for is_q, ps in zip((True, False), psqk):
                    bcol = j if is_q else 8 + j
                    nc.scalar.activation(
                        ps[:], ps[:], AF.Exp, bias=qbias_sb[:, bcol : bcol + 1]
                    )
                sps = []
                for ps in psqk:
                    sp = spp.tile([128, N], BF16, tag="sp")
                    nc.scalar.activation(sp[:], ps[:], AF.Ln, bias=1.0)
                    sps.append(sp)
                out = []
                for is_q, sp in zip((True, False), sps):
                    pool = q2p if is_q else k2p
                    cs = csq_sb if is_q else csk_sb
                    tiles = []
                    for hh in range(2):  # head 2j+hh
                        dup = pool.tile([128, N], BF16, tag="d")
                        nc.sync.dma_start(
                            out=dup[0:64, :], in_=sp[hh * 64 : hh * 64 + 64, :]
                        )
                        nc.sync.dma_start(
                            out=dup[64:128, :], in_=sp[hh * 64 : hh * 64 + 64, :]
                        )
                        nc.vector.tensor_mul(out=dup[:], in0=dup[:], in1=cs[:])
                        tiles.append(dup)
                    out.append(tiles)
                return out  # [q_tiles, k_tiles]

            et_tiles = {}

            def dots(h, q2, k2):
                ets = []
                for i in range(MC_):
                    ps = psA.tile([128, N], F32, tag="ps")
                    for hlf in range(2):
                        nc.tensor.matmul(
                            ps[:, hlf * 512 : (hlf + 1) * 512],
                            lhsT=k2[:, i * 128 : (i + 1) * 128],
                            rhs=q2[:, hlf * 512 : (hlf + 1) * 512],
                            start=True,
                            stop=True,
                        )
                    et = etp.tile([128, N], BF16, tag="et")
                    nc.scalar.activation(et[:], ps[:], AF.Exp, scale=SCALE)
                    ets.append(et)
                et_tiles[h] = ets

            fin_state = {}

            def stage2_mm(h):
                """O_un^T = V'.T @ E^T; drain PSUM; start the 1/L chain."""
                ets = et_tiles.pop(h)
                po = psOun.tile([128, N], F32, tag="oun")
                for i in range(MC_):
                    for hlf in range(2):
                        nc.tensor.matmul(
                            po[0:65, hlf * 512 : (hlf + 1) * 512],
                            lhsT=vp[:, i, h * 65 : (h + 1) * 65],
                            rhs=ets[i][:, hlf * 512 : (hlf + 1) * 512],
                            start=(i == 0),
                            stop=(i == MC_ - 1),
                        )
                po_sb = posbp.tile([128, N], F32, tag="posb")
                nc.vector.tensor_copy(out=po_sb[0:65, :], in_=po[0:65, :])
                drs = drsp.tile([1, N], F32, tag="drs")
                nc.sync.dma_start(out=drs[:], in_=po_sb[64:65, :])
                lcol = reclp.tile([128, 8], F32, tag="lcol")
                nc.sync.dma_start(
                    out=lcol[:], in_=drs.rearrange("o (c p) -> (o p) c", p=128)
                )
                lcolr = reclp.tile([128, 8], F32, tag="lcolr")
                nc.vector.reciprocal(out=lcolr[:], in_=lcol[:])
                drs2 = drsp.tile([1, N], F32, tag="drs2")
                nc.sync.dma_start(
                    out=drs2.rearrange("o (c p) -> (o p) c", p=128), in_=lcolr[:]
                )
                lbc = recbcp.tile([128, N], F32, tag="lbc")
                nc.sync.dma_start(out=lbc[0:64, :], in_=bcast(drs2[0:1, :], 64))
                fin_state[h] = (po_sb, lbc)

            def stage2_fin(h):
                """One j-step later: multiply O_un by the broadcast 1/L."""
                po_sb, lbc = fin_state.pop(h)
                if h % 2 == 0:
                    nc.vector.tensor_mul(
                        out=ot_sb[0:64, h // 2, :],
                        in0=po_sb[0:64, :],
                        in1=lbc[0:64, :],
                    )
                else:
                    # DVE can't shift partitions; bounce via SBUF + DMA
                    otmp = reclp.tile([128, N], BF16, tag="otmp")
                    nc.vector.tensor_mul(
                        out=otmp[0:64, :], in0=po_sb[0:64, :], in1=lbc[0:64, :]
                    )
                    nc.sync.dma_start(
                        out=ot_sb[64:128, h // 2, :], in_=otmp[0:64, :]
                    )

            # ---- lag-2 pipelined attention --------------------------
            pairs = {}
            pairs[0] = qk_pair(0)
            pairs[1] = qk_pair(1)
            for j in range(EC_Q):
                qj, kj = pairs.pop(j)
                dots(2 * j, qj[0], kj[0])
                if j >= 1:
                    stage2_mm(2 * j - 2)
                dots(2 * j + 1, qj[1], kj[1])
                if j >= 1:
                    stage2_mm(2 * j - 1)
                if j + 2 < EC_Q:
                    pairs[j + 2] = qk_pair(j + 2)
                if j >= 2:
                    stage2_fin(2 * j - 4)
                    stage2_fin(2 * j - 3)
            stage2_mm(14)
            stage2_mm(15)
            for h in (12, 13, 14, 15):
                stage2_fin(h)

            # ---- Phase F: output projection -------------------------
            for c in range(NC_):
                ps = psA.tile([128, N], F32, tag="ps")
                for t in range(DT_):
                    for hlf in range(2):
                        nc.tensor.matmul(
                            ps[:, hlf * 512 : (hlf + 1) * 512],
                            lhsT=ot_sb[:, t, c * 128 : (c + 1) * 128],
                            rhs=wout_sb[:, t, hlf * 512 : (hlf + 1) * 512],
                            start=(t == 0),
                            stop=(t == DT_ - 1),
                        )
                o_t = outp.tile([128, D], F32, tag="of")
                nc.vector.tensor_add(out=o_t[:], in0=ps[:], in1=bout_sb[:])
                nc.sync.dma_start(out=out_d[c * 128 : (c + 1) * 128, :], in_=o_t[:])


_NC_CACHE = {}


def _get_nc():
    if "nc" not in _NC_CACHE:
        # NOTE: do NOT reorder activation tables to pin
        # natural_log_exp_and_others - that table has no valid runtime
        # payload on HW (all-NaN results).  The emit order batches
        # same-function ACT ops to minimize table switches instead.
        nc = bacc.Bacc(
            "TRN2",
            target_bir_lowering=False,
            debug=False,
            enable_asserts=False,
            num_devices=8,
        )
        with tile.TileContext(nc) as tc:
            _emit(tc)
        nc.compile()
        _NC_CACHE["nc"] = nc
    return _NC_CACHE["nc"]


def _trace_ok():
    try:
        from antenv.axon_hooks import get_axon_ntff_profile_hook

        return get_axon_ntff_profile_hook() is not None
    except Exception:
        return False


def kernel(**inputs):
    bf = ml_dtypes.bfloat16
    x = np.ascontiguousarray(np.asarray(inputs["x"], dtype=np.float32))
    freqs = np.asarray(inputs["freqs"], dtype=np.float32)[0]
    fbias = np.asarray(inputs["bias"], dtype=np.float32)[0]
    g = np.asarray(inputs["ln_gamma"], dtype=np.float32)
    be = np.asarray(inputs["ln_beta"], dtype=np.float32)
    w_qk = np.asarray(inputs["w_qk"], dtype=np.float32)
    w_v = np.asarray(inputs["w_v"], dtype=np.float32)
    w_out = np.asarray(inputs["w_out"], dtype=np.float32)
    b_out = np.asarray(inputs["b_out"], dtype=np.float32)

    wqk_s = np.ascontiguousarray((w_qk * g[:, None]).astype(bf))
    wv_s = np.ascontiguousarray((w_v * g[:, None]).astype(bf))
    wout_b = np.ascontiguousarray(w_out.astype(bf))
    qb = be @ w_qk  # [2048]; beta contribution to q/k pre-activation
    vb = (be @ w_v).astype(np.float32)[None, :]
    qbias = np.ascontiguousarray(qb.reshape(16, 128).T.astype(np.float32))
    csq = np.ascontiguousarray(
        np.concatenate([np.cos(freqs).T, np.sin(freqs).T], axis=0).astype(bf)
    )
    fb = freqs + fbias
    csk = np.ascontiguousarray(
        np.concatenate([np.cos(fb).T, np.sin(fb).T], axis=0).astype(bf)
    )
    bout = np.ascontiguousarray(b_out[None, :].astype(np.float32))

    shared = dict(
        wqk=wqk_s, wv=wv_s, wout=wout_b, csq=csq, csk=csk,
        qbias=qbias, vbias=vb, bout=bout,
    )
    in_maps = [dict(x=np.ascontiguousarray(x[i]), **shared) for i in range(B)]

    nc = _get_nc()
    want_trace = bool(int(os.environ.get("KERNEL_TRACE", "0")))
    res = run_bass_kernel_spmd(
        nc,
        in_maps,
        core_ids=list(range(B)),
        trace=want_trace and _trace_ok(),
    )
    out = np.stack([res.results[i]["out"] for i in range(B)], axis=0)
    if getattr(res, "exec_time_ns", None):
        kernel.last_exec_time_ns = res.exec_time_ns
    kernel.last_results = res
    return out
